# revision 18
# baseline (speedup 1.0000x reference)
"""Multi-head attention (B=2, S=2048, D=768, H=12) on 8 TRN2 NeuronCores.

Sharding: data-parallel over batch x tensor-parallel over heads.
  core c -> batch c//4, heads 3*(c%4) .. 3*(c%4)+2
Each core computes its 3 heads end-to-end plus the partial output
projection (its 192 rows of w_proj). Host sums the 4 partials per batch
and adds b_proj. No cross-core collectives needed.

Device kernel layout notes (per core):
  xT   (768, 2048)  = x[b]^T           -> contraction dim on partitions
  wqk  (768, 384)   = [Wq_heads*SCALE | Wk_heads], cols q0 q1 q2 k0 k1 k2
  Q^T/K^T tiles: tq0=[q0;q1] (128p), tq1=[q2] (64p), tk0=[k0;k1], tk1=[k2]
    so each head's Q^T and K^T slices share a base partition (0 or 64),
    which the matmul tile_position inference requires.
  V1   per k-chunk [128, 195]: cols [v_h0 1 v_h1 1 v_h2 1] (ones col per head)
  Scores are computed transposed (k on partitions): S^T[k, qb] so that
  exp(S^T) feeds the AV matmul directly (no transpose of the softmax).
  AV: O'[65, qb] = [V_h|1]^T @ exp(S^T chunk); row 64 = softmax denominator.
  Normalize: recip(denom) broadcast over partitions via a K=1 ones-matmul.
"""

from contextlib import ExitStack

import numpy as np

import concourse.bass as bass
import concourse.mybir as mybir
import concourse.tile as tile
from concourse import bacc
from concourse.bass_utils import run_bass_kernel_spmd

B, S, D = 2, 2048, 768
H, HD = 12, 64
SCALE = HD**-0.5
NCORES = 8
HPC = 3  # heads per core
P = 128
KD = D // P  # 6 chunks of contraction dim for projections
QB = 512  # query block (free dim of score matmuls)
NQB = S // QB  # 4
NKC = S // P  # 16 key chunks
GS = 2  # key chunks per exp group ([128, GS*QB] activation)
F32 = mybir.dt.float32
EXP = mybir.ActivationFunctionType.Exp

_CACHE = {}


def _build_nc():
    nc = bacc.Bacc("TRN2", target_bir_lowering=False, debug=False)
    xT_d = nc.dram_tensor("xT", [D, S], F32, kind="ExternalInput").ap()
    wqk_d = nc.dram_tensor("wqk", [D, 2 * HPC * HD], F32, kind="ExternalInput").ap()
    bqk_d = nc.dram_tensor("bqk", [P, 4], F32, kind="ExternalInput").ap()
    wv_d = nc.dram_tensor("wv", [D, HPC * HD], F32, kind="ExternalInput").ap()
    bv_d = nc.dram_tensor("bv", [HD, HPC], F32, kind="ExternalInput").ap()
    wp_d = nc.dram_tensor("wp", [HPC * HD, D], F32, kind="ExternalInput").ap()
    out_d = nc.dram_tensor("out", [S, D], F32, kind="ExternalOutput").ap()

    with tile.TileContext(nc) as tc, ExitStack() as ctx:
        const = ctx.enter_context(tc.tile_pool(name="const", bufs=1))
        stage = ctx.enter_context(tc.tile_pool(name="stage", bufs=2))
        es_pool = ctx.enter_context(tc.tile_pool(name="es", bufs=3))
        ot_pool = ctx.enter_context(tc.tile_pool(name="ot", bufs=2))
        rc_pool = ctx.enter_context(tc.tile_pool(name="rc", bufs=2))
        out_pool = ctx.enter_context(tc.tile_pool(name="outsb", bufs=3))
        qkv_ctx = ExitStack()
        ps_qkt = qkv_ctx.enter_context(tc.tile_pool(name="ps_qkt", bufs=3, space="PSUM"))
        ps_v = qkv_ctx.enter_context(tc.tile_pool(name="ps_v", bufs=3, space="PSUM"))

        # ---- load inputs ----
        # Matmul-input tiles are staged DMA -> stage tile -> DVE copy, so
        # every matmul's input-ready dependency is a single DVE semaphore
        # (walrus allows only ONE sync-wait on a Matmult instruction; direct
        # DMA deps would attach 1-2 HW-DGE queue semaphores per input).
        def staged_load(dst, src_ap):
            st_t = stage.tile(list(dst.shape), F32, tag="stage")
            nc.gpsimd.dma_start(out=st_t[:], in_=src_ap)
            nc.vector.tensor_copy(dst[:], st_t[:])

        xt = []
        wqk_sb = []
        wv_sb = []
        for i in range(KD):
            t = const.tile([P, S], F32, tag=f"xt{i}")
            staged_load(t, xT_d[i * P : (i + 1) * P, :])
            xt.append(t)
            t = const.tile([P, 2 * HPC * HD], F32, tag=f"wqk{i}")
            staged_load(t, wqk_d[i * P : (i + 1) * P, :])
            wqk_sb.append(t)
            t = const.tile([P, HPC * HD], F32, tag=f"wv{i}")
            staged_load(t, wv_d[i * P : (i + 1) * P, :])
            wv_sb.append(t)
        wp_sb = []
        for h in range(HPC):
            t = const.tile([HD, D], F32, tag=f"wp{h}")
            staged_load(t, wp_d[h * HD : (h + 1) * HD, :])
            wp_sb.append(t)
        bqk_sb = const.tile([P, 4], F32, tag="bqk")
        nc.gpsimd.dma_start(out=bqk_sb[:], in_=bqk_d[:, :])
        bv_sb = const.tile([HD, HPC], F32, tag="bv")
        nc.gpsimd.dma_start(out=bv_sb[:], in_=bv_d[:, :])
        # ones row lives at partition HD (=64) so the K=1 broadcast matmul's
        # operands (ones row, reciprocal row) sit at the same base partition
        # as the denominator row of the AV psum they derive from — engine
        # lanes are partition-wired, so ops must not shift partitions.
        ones_sb = const.tile([HD + 1, HD], F32, tag="ones")
        nc.vector.memset(ones_sb[:], 1.0)

        # ---- Q^T / K^T projection into 4 aligned tiles ----
        tq0 = const.tile([P, S], F32, tag="tq0")  # [q_h0; q_h1]
        tq1 = const.tile([HD, S], F32, tag="tq1")  # [q_h2]
        tk0 = const.tile([P, S], F32, tag="tk0")  # [k_h0; k_h1]
        tk1 = const.tile([HD, S], F32, tag="tk1")  # [k_h2]
        # (dst tile, wqk col start, width, bias col)
        m_specs = [
            (tq0, 0, P, 0),
            (tq1, P, HD, 1),
            (tk0, P + HD, P, 2),
            (tk1, 2 * P + HD, HD, 3),
        ]
        for dst, c0, w, bcol in m_specs:
            for nb in range(NQB):
                pq = ps_qkt.tile([P, QB], F32, tag="qkt")
                for k in range(KD):
                    nc.tensor.matmul(
                        pq[:w],
                        wqk_sb[k][:, c0 : c0 + w],
                        xt[k][:, nb * QB : (nb + 1) * QB],
                        start=(k == 0),
                        stop=(k == KD - 1),
                    )
                nc.vector.tensor_scalar_add(
                    dst[:, nb * QB : (nb + 1) * QB],
                    pq[:w],
                    bqk_sb[:w, bcol : bcol + 1],
                )

        # per-head (tile, partition offset) for Q^T and K^T
        q_loc = [(tq0, 0), (tq0, HD), (tq1, 0)]
        k_loc = [(tk0, 0), (tk0, HD), (tk1, 0)]

        # ---- V projection into [128, 195] per key chunk (ones col per head) ----
        v1 = []
        for st in range(NKC):
            pv = ps_v.tile([P, HPC * HD], F32, tag="v")
            for k in range(KD):
                nc.tensor.matmul(
                    pv[:],
                    xt[k][:, st * P : (st + 1) * P],
                    wv_sb[k][:],
                    start=(k == 0),
                    stop=(k == KD - 1),
                )
            t = const.tile([P, HPC * (HD + 1)], F32, tag=f"v1_{st}")
            nc.vector.memset(t[:], 1.0)
            for h in range(HPC):
                nc.vector.tensor_copy(
                    t[:, h * (HD + 1) : h * (HD + 1) + HD],
                    pv[:, h * HD : (h + 1) * HD],
                )
            v1.append(t)

        qkv_ctx.close()
        attn_ctx = ExitStack()
        ps_s = attn_ctx.enter_context(tc.tile_pool(name="ps_s", bufs=2, space="PSUM"))
        ps_o = attn_ctx.enter_context(tc.tile_pool(name="ps_o", bufs=2, space="PSUM"))
        ps_r = attn_ctx.enter_context(tc.tile_pool(name="ps_r", bufs=1, space="PSUM"))
        ps_p = attn_ctx.enter_context(tc.tile_pool(name="ps_p", bufs=1, space="PSUM"))

        # ---- attention + partial output projection ----
        for qb in range(NQB):
            ots = []
            for h in range(HPC):
                qt, qo = q_loc[h]
                kt, ko = k_loc[h]
                q_ap = qt[qo : qo + HD, qb * QB : (qb + 1) * QB]
                po = ps_o.tile([HD + 1, QB], F32, tag="o")
                for g in range(NKC // GS):
                    ps = ps_s.tile([P, GS * QB], F32, tag="s")
                    for j in range(GS):
                        kc = g * GS + j
                        nc.tensor.matmul(
                            ps[:, j * QB : (j + 1) * QB],
                            kt[ko : ko + HD, kc * P : (kc + 1) * P],
                            q_ap,
                            start=True,
                            stop=True,
                        )
                    es = es_pool.tile([P, GS * QB], F32, tag="es")
                    nc.scalar.activation(es[:], ps[:], EXP)
                    for j in range(GS):
                        kc = g * GS + j
                        nc.tensor.matmul(
                            po[:],
                            v1[kc][:, h * (HD + 1) : (h + 1) * (HD + 1)],
                            es[:, j * QB : (j + 1) * QB],
                            start=(kc == 0),
                            stop=(kc == NKC - 1),
                        )
                # normalize: out_h = O'[0:64] * (1/denom) + bv_h
                # denom row -> SBUF (partition-aligned copy), broadcast the
                # raw denominator over 64 partitions via a K=1 ones-matmul,
                # then take the reciprocal at base partition 0
                # (reciprocal_approx_fast miscomputes at nonzero base
                # partition on HW, so it must run after the broadcast).
                dsb = rc_pool.tile([HD + 1, QB], F32, tag="rc")
                nc.vector.tensor_copy(dsb[HD : HD + 1, :], po[HD : HD + 1, :])
                pr = ps_r.tile([HD, QB], F32, tag="r")
                nc.tensor.matmul(
                    pr[:],
                    ones_sb[HD : HD + 1, :],
                    dsb[HD : HD + 1, :],
                    start=True,
                    stop=True,
                )
                rbs = rc_pool.tile([HD, QB], F32, tag="rbs")
                nc.vector.tensor_copy(rbs[:], pr[:])
                rb = rc_pool.tile([HD, QB], F32, tag="rb")
                nc.vector.reciprocal_approx_fast(rb[:], rbs[:])
                ot = ot_pool.tile([HD, QB], F32, tag=f"ot{h}")
                nc.vector.tensor_mul(ot[:], po[0:HD, :], rb[:])
                nc.vector.tensor_scalar_add(ot[:], ot[:], bv_sb[:, h : h + 1])
                ots.append(ot)

            # partial output projection for the 4 row tiles of this q block
            for t_i in range(QB // P):
                st = qb * (QB // P) + t_i
                outsb = out_pool.tile([P, D], F32, tag="outsb")
                for nb in range(2):
                    pp = ps_p.tile([P, D // 2], F32, tag="p")
                    for h in range(HPC):
                        nc.tensor.matmul(
                            pp[:],
                            ots[h][:, t_i * P : (t_i + 1) * P],
                            wp_sb[h][:, nb * (D // 2) : (nb + 1) * (D // 2)],
                            start=(h == 0),
                            stop=(h == HPC - 1),
                        )
                    nc.vector.tensor_copy(
                        outsb[:, nb * (D // 2) : (nb + 1) * (D // 2)], pp[:]
                    )
                nc.gpsimd.dma_start(out=out_d[st * P : (st + 1) * P, :], in_=outsb[:])

        attn_ctx.close()

    nc.compile()
    return nc


def get_nc():
    if "nc" not in _CACHE:
        _CACHE["nc"] = _build_nc()
    return _CACHE["nc"]


def shard_inputs(x, w_qkv, b_qkv, w_proj):
    x = np.asarray(x, np.float32)
    w_qkv = np.asarray(w_qkv, np.float32)
    b_qkv = np.asarray(b_qkv, np.float32)
    w_proj = np.asarray(w_proj, np.float32)
    Wq, Wk, Wv = w_qkv[:, :D], w_qkv[:, D : 2 * D], w_qkv[:, 2 * D :]
    bq, bk, bv = b_qkv[:D], b_qkv[D : 2 * D], b_qkv[2 * D :]
    in_maps = []
    for c in range(NCORES):
        b = c // 4
        lo = HD * HPC * (c % 4)
        sl = slice(lo, lo + HPC * HD)
        bq_s = bq[sl] * SCALE
        bk_s = bk[sl]
        bias4 = np.zeros((P, 4), np.float32)
        bias4[:, 0] = bq_s[0:P]
        bias4[:HD, 1] = bq_s[P : P + HD]
        bias4[:, 2] = bk_s[0:P]
        bias4[:HD, 3] = bk_s[P : P + HD]
        in_maps.append(
            {
                "xT": np.ascontiguousarray(x[b].T),
                "wqk": np.ascontiguousarray(
                    np.concatenate([Wq[:, sl] * SCALE, Wk[:, sl]], axis=1)
                ),
                "bqk": bias4,
                "wv": np.ascontiguousarray(Wv[:, sl]),
                "bv": np.ascontiguousarray(bv[sl].reshape(HPC, HD).T),
                "wp": np.ascontiguousarray(w_proj[sl, :]),
            }
        )
    return in_maps


def assemble(outs, b_proj):
    b_proj = np.asarray(b_proj, np.float32)
    y = np.empty((B, S, D), np.float32)
    for b in range(B):
        acc = outs[4 * b].astype(np.float32)
        for i in range(1, 4):
            acc = acc + outs[4 * b + i]
        y[b] = acc + b_proj
    return y


def run(inputs, trace=False, **kw):
    nc = get_nc()
    in_maps = shard_inputs(
        inputs["x"], inputs["w_qkv"], inputs["b_qkv"], inputs["w_proj"]
    )
    res = run_bass_kernel_spmd(
        nc, in_maps, core_ids=list(range(NCORES)), trace=trace, **kw
    )
    outs = [r["out"] for r in res.results]
    return assemble(outs, inputs["b_proj"]), res


def kernel(x, w_qkv, b_qkv, w_proj, b_proj):
    y, _ = run(
        {"x": x, "w_qkv": w_qkv, "b_qkv": b_qkv, "w_proj": w_proj, "b_proj": b_proj}
    )
    return y


# revision 19
# speedup vs baseline: 1.8510x; 1.8510x over previous
"""Multi-head attention (B=2, S=2048, D=768, H=12) on 8 TRN2 NeuronCores.

Sharding: data-parallel over batch x tensor-parallel over heads.
  core c -> batch c//4, heads 3*(c%4) .. 3*(c%4)+2
Each core computes its 3 heads end-to-end plus the partial output
projection (its 192 rows of w_proj). Host sums the 4 partials per batch
and adds b_proj. No cross-core collectives needed.

Device kernel layout notes (per core):
  xT   (768, 2048)  = x[b]^T           -> contraction dim on partitions
  wqk  (768, 384)   = [Wq_heads*SCALE | Wk_heads], cols q0 q1 q2 k0 k1 k2
  Q^T/K^T tiles: tq0=[q0;q1] (128p), tq1=[q2] (64p), tk0=[k0;k1], tk1=[k2]
    so each head's Q^T and K^T slices share a base partition (0 or 64),
    which the matmul tile_position inference requires.
  V1   per k-chunk [128, 195]: cols [v_h0 1 v_h1 1 v_h2 1] (ones col per head)
  Scores are computed transposed (k on partitions): S^T[k, qb] so that
  exp(S^T) feeds the AV matmul directly (no transpose of the softmax).
  AV: O'[65, qb] = [V_h|1]^T @ exp(S^T chunk); row 64 = softmax denominator.
  Normalize: recip(denom) broadcast over partitions via a K=1 ones-matmul.
"""

from contextlib import ExitStack

import numpy as np

import concourse.bass as bass
import concourse.mybir as mybir
import concourse.tile as tile
from concourse import bacc
from concourse.bass_utils import run_bass_kernel_spmd

B, S, D = 2, 2048, 768
H, HD = 12, 64
SCALE = HD**-0.5
NCORES = 8
HPC = 3  # heads per core
P = 128
KD = D // P  # 6 chunks of contraction dim for projections
QB = 512  # query block (free dim of score matmuls)
NQB = S // QB  # 4
NKC = S // P  # 16 key chunks
GS = 2  # key chunks per exp group ([128, GS*QB] activation)
F32 = mybir.dt.float32
BF16 = mybir.dt.bfloat16
EXP = mybir.ActivationFunctionType.Exp

_CACHE = {}


def _build_nc():
    nc = bacc.Bacc("TRN2", target_bir_lowering=False, debug=False)
    xT_d = nc.dram_tensor("xT", [D, S], F32, kind="ExternalInput").ap()
    wqk_d = nc.dram_tensor("wqk", [D, 2 * HPC * HD], F32, kind="ExternalInput").ap()
    bqk_d = nc.dram_tensor("bqk", [P, 4], F32, kind="ExternalInput").ap()
    wv_d = nc.dram_tensor("wv", [D, HPC * HD], F32, kind="ExternalInput").ap()
    bv_d = nc.dram_tensor("bv", [HD, HPC], F32, kind="ExternalInput").ap()
    wp_d = nc.dram_tensor("wp", [HPC * HD, D], F32, kind="ExternalInput").ap()
    out_d = nc.dram_tensor("out", [S, D], F32, kind="ExternalOutput").ap()

    with tile.TileContext(nc) as tc, ExitStack() as ctx:
        const = ctx.enter_context(tc.tile_pool(name="const", bufs=1))
        stage = ctx.enter_context(tc.tile_pool(name="stage", bufs=2))
        es_pool = ctx.enter_context(tc.tile_pool(name="es", bufs=3))
        ot_pool = ctx.enter_context(tc.tile_pool(name="ot", bufs=2))
        rc_pool = ctx.enter_context(tc.tile_pool(name="rc", bufs=2))
        out_pool = ctx.enter_context(tc.tile_pool(name="outsb", bufs=3))
        qkv_ctx = ExitStack()
        ps_qkt = qkv_ctx.enter_context(tc.tile_pool(name="ps_qkt", bufs=3, space="PSUM"))
        ps_v = qkv_ctx.enter_context(tc.tile_pool(name="ps_v", bufs=3, space="PSUM"))

        # ---- load inputs ----
        # Matmul-input tiles are staged DMA -> stage tile -> DVE copy, so
        # every matmul's input-ready dependency is a single DVE semaphore
        # (walrus allows only ONE sync-wait on a Matmult instruction; direct
        # DMA deps would attach 1-2 HW-DGE queue semaphores per input).
        def staged_load(dst, src_ap):
            st_t = stage.tile(list(dst.shape), F32, tag="stage")
            nc.gpsimd.dma_start(out=st_t[:], in_=src_ap)
            nc.vector.tensor_copy(dst[:], st_t[:])

        xt = []
        wqk_sb = []
        wv_sb = []
        for i in range(KD):
            t = const.tile([P, S], BF16, tag=f"xt{i}")
            staged_load(t, xT_d[i * P : (i + 1) * P, :])
            xt.append(t)
            t = const.tile([P, 2 * HPC * HD], BF16, tag=f"wqk{i}")
            staged_load(t, wqk_d[i * P : (i + 1) * P, :])
            wqk_sb.append(t)
            t = const.tile([P, HPC * HD], BF16, tag=f"wv{i}")
            staged_load(t, wv_d[i * P : (i + 1) * P, :])
            wv_sb.append(t)
        wp_sb = []
        for h in range(HPC):
            t = const.tile([HD, D], BF16, tag=f"wp{h}")
            staged_load(t, wp_d[h * HD : (h + 1) * HD, :])
            wp_sb.append(t)
        bqk_sb = const.tile([P, 4], F32, tag="bqk")
        nc.gpsimd.dma_start(out=bqk_sb[:], in_=bqk_d[:, :])
        bv_sb = const.tile([HD, HPC], F32, tag="bv")
        nc.gpsimd.dma_start(out=bv_sb[:], in_=bv_d[:, :])
        # ones row lives at partition HD (=64) so the K=1 broadcast matmul's
        # operands (ones row, reciprocal row) sit at the same base partition
        # as the denominator row of the AV psum they derive from — engine
        # lanes are partition-wired, so ops must not shift partitions.
        ones_sb = const.tile([HD + 1, HD], F32, tag="ones")
        nc.vector.memset(ones_sb[:], 1.0)

        # ---- Q^T / K^T projection into 4 aligned tiles ----
        tq0 = const.tile([P, S], BF16, tag="tq0")  # [q_h0; q_h1]
        tq1 = const.tile([HD, S], BF16, tag="tq1")  # [q_h2]
        tk0 = const.tile([P, S], BF16, tag="tk0")  # [k_h0; k_h1]
        tk1 = const.tile([HD, S], BF16, tag="tk1")  # [k_h2]
        # (dst tile, wqk col start, width, bias col)
        m_specs = [
            (tq0, 0, P, 0),
            (tq1, P, HD, 1),
            (tk0, P + HD, P, 2),
            (tk1, 2 * P + HD, HD, 3),
        ]
        for dst, c0, w, bcol in m_specs:
            for nb in range(NQB):
                pq = ps_qkt.tile([P, QB], F32, tag="qkt")
                for k in range(KD):
                    nc.tensor.matmul(
                        pq[:w],
                        wqk_sb[k][:, c0 : c0 + w],
                        xt[k][:, nb * QB : (nb + 1) * QB],
                        start=(k == 0),
                        stop=(k == KD - 1),
                    )
                nc.vector.tensor_scalar_add(
                    dst[:, nb * QB : (nb + 1) * QB],
                    pq[:w],
                    bqk_sb[:w, bcol : bcol + 1],
                )

        # per-head (tile, partition offset) for Q^T and K^T
        q_loc = [(tq0, 0), (tq0, HD), (tq1, 0)]
        k_loc = [(tk0, 0), (tk0, HD), (tk1, 0)]

        # ---- V projection into [128, 195] per key chunk (ones col per head) ----
        v1 = []
        for st in range(NKC):
            pv = ps_v.tile([P, HPC * HD], F32, tag="v")
            for k in range(KD):
                nc.tensor.matmul(
                    pv[:],
                    xt[k][:, st * P : (st + 1) * P],
                    wv_sb[k][:],
                    start=(k == 0),
                    stop=(k == KD - 1),
                )
            t = const.tile([P, HPC * (HD + 1)], BF16, tag=f"v1_{st}")
            nc.vector.memset(t[:], 1.0)
            for h in range(HPC):
                nc.vector.tensor_copy(
                    t[:, h * (HD + 1) : h * (HD + 1) + HD],
                    pv[:, h * HD : (h + 1) * HD],
                )
            v1.append(t)

        qkv_ctx.close()
        attn_ctx = ExitStack()
        ps_s = attn_ctx.enter_context(tc.tile_pool(name="ps_s", bufs=2, space="PSUM"))
        ps_o = attn_ctx.enter_context(tc.tile_pool(name="ps_o", bufs=2, space="PSUM"))
        ps_r = attn_ctx.enter_context(tc.tile_pool(name="ps_r", bufs=1, space="PSUM"))
        ps_p = attn_ctx.enter_context(tc.tile_pool(name="ps_p", bufs=1, space="PSUM"))

        # ---- attention + partial output projection ----
        for qb in range(NQB):
            ots = []
            for h in range(HPC):
                qt, qo = q_loc[h]
                kt, ko = k_loc[h]
                q_ap = qt[qo : qo + HD, qb * QB : (qb + 1) * QB]
                po = ps_o.tile([HD + 1, QB], F32, tag="o")
                for g in range(NKC // GS):
                    ps = ps_s.tile([P, GS * QB], F32, tag="s")
                    for j in range(GS):
                        kc = g * GS + j
                        nc.tensor.matmul(
                            ps[:, j * QB : (j + 1) * QB],
                            kt[ko : ko + HD, kc * P : (kc + 1) * P],
                            q_ap,
                            start=True,
                            stop=True,
                        )
                    es = es_pool.tile([P, GS * QB], BF16, tag="es")
                    nc.scalar.activation(es[:], ps[:], EXP)
                    for j in range(GS):
                        kc = g * GS + j
                        nc.tensor.matmul(
                            po[:],
                            v1[kc][:, h * (HD + 1) : (h + 1) * (HD + 1)],
                            es[:, j * QB : (j + 1) * QB],
                            start=(kc == 0),
                            stop=(kc == NKC - 1),
                        )
                # normalize: out_h = O'[0:64] * (1/denom) + bv_h
                # denom row -> SBUF (partition-aligned copy), broadcast the
                # raw denominator over 64 partitions via a K=1 ones-matmul,
                # then take the reciprocal at base partition 0
                # (reciprocal_approx_fast miscomputes at nonzero base
                # partition on HW, so it must run after the broadcast).
                dsb = rc_pool.tile([HD + 1, QB], F32, tag="rc")
                nc.vector.tensor_copy(dsb[HD : HD + 1, :], po[HD : HD + 1, :])
                pr = ps_r.tile([HD, QB], F32, tag="r")
                nc.tensor.matmul(
                    pr[:],
                    ones_sb[HD : HD + 1, :],
                    dsb[HD : HD + 1, :],
                    start=True,
                    stop=True,
                )
                rbs = rc_pool.tile([HD, QB], F32, tag="rbs")
                nc.vector.tensor_copy(rbs[:], pr[:])
                rb = rc_pool.tile([HD, QB], F32, tag="rb")
                nc.vector.reciprocal_approx_fast(rb[:], rbs[:])
                ot = ot_pool.tile([HD, QB], BF16, tag=f"ot{h}")
                nc.vector.tensor_mul(ot[:], po[0:HD, :], rb[:])
                nc.vector.tensor_scalar_add(ot[:], ot[:], bv_sb[:, h : h + 1])
                ots.append(ot)

            # partial output projection for the 4 row tiles of this q block
            for t_i in range(QB // P):
                st = qb * (QB // P) + t_i
                outsb = out_pool.tile([P, D], F32, tag="outsb")
                for nb in range(2):
                    pp = ps_p.tile([P, D // 2], F32, tag="p")
                    for h in range(HPC):
                        nc.tensor.matmul(
                            pp[:],
                            ots[h][:, t_i * P : (t_i + 1) * P],
                            wp_sb[h][:, nb * (D // 2) : (nb + 1) * (D // 2)],
                            start=(h == 0),
                            stop=(h == HPC - 1),
                        )
                    nc.vector.tensor_copy(
                        outsb[:, nb * (D // 2) : (nb + 1) * (D // 2)], pp[:]
                    )
                nc.gpsimd.dma_start(out=out_d[st * P : (st + 1) * P, :], in_=outsb[:])

        attn_ctx.close()

    nc.compile()
    return nc


def get_nc():
    if "nc" not in _CACHE:
        _CACHE["nc"] = _build_nc()
    return _CACHE["nc"]


def shard_inputs(x, w_qkv, b_qkv, w_proj):
    x = np.asarray(x, np.float32)
    w_qkv = np.asarray(w_qkv, np.float32)
    b_qkv = np.asarray(b_qkv, np.float32)
    w_proj = np.asarray(w_proj, np.float32)
    Wq, Wk, Wv = w_qkv[:, :D], w_qkv[:, D : 2 * D], w_qkv[:, 2 * D :]
    bq, bk, bv = b_qkv[:D], b_qkv[D : 2 * D], b_qkv[2 * D :]
    in_maps = []
    for c in range(NCORES):
        b = c // 4
        lo = HD * HPC * (c % 4)
        sl = slice(lo, lo + HPC * HD)
        bq_s = bq[sl] * SCALE
        bk_s = bk[sl]
        bias4 = np.zeros((P, 4), np.float32)
        bias4[:, 0] = bq_s[0:P]
        bias4[:HD, 1] = bq_s[P : P + HD]
        bias4[:, 2] = bk_s[0:P]
        bias4[:HD, 3] = bk_s[P : P + HD]
        in_maps.append(
            {
                "xT": np.ascontiguousarray(x[b].T),
                "wqk": np.ascontiguousarray(
                    np.concatenate([Wq[:, sl] * SCALE, Wk[:, sl]], axis=1)
                ),
                "bqk": bias4,
                "wv": np.ascontiguousarray(Wv[:, sl]),
                "bv": np.ascontiguousarray(bv[sl].reshape(HPC, HD).T),
                "wp": np.ascontiguousarray(w_proj[sl, :]),
            }
        )
    return in_maps


def assemble(outs, b_proj):
    b_proj = np.asarray(b_proj, np.float32)
    y = np.empty((B, S, D), np.float32)
    for b in range(B):
        acc = outs[4 * b].astype(np.float32)
        for i in range(1, 4):
            acc = acc + outs[4 * b + i]
        y[b] = acc + b_proj
    return y


def run(inputs, trace=False, **kw):
    nc = get_nc()
    in_maps = shard_inputs(
        inputs["x"], inputs["w_qkv"], inputs["b_qkv"], inputs["w_proj"]
    )
    res = run_bass_kernel_spmd(
        nc, in_maps, core_ids=list(range(NCORES)), trace=trace, **kw
    )
    outs = [r["out"] for r in res.results]
    return assemble(outs, inputs["b_proj"]), res


def kernel(x, w_qkv, b_qkv, w_proj, b_proj):
    y, _ = run(
        {"x": x, "w_qkv": w_qkv, "b_qkv": b_qkv, "w_proj": w_proj, "b_proj": b_proj}
    )
    return y
